# revision 12
# baseline (speedup 1.0000x reference)
"""GQA attention (B=2, L=2048, HID=2048, 32 Q heads / 8 KV heads) on 8 TRN2 cores.

Sharding: data-parallel on batch (2) x tensor-parallel on heads (4).
Core c: batch b = c//4, TP rank r = c%4 owns q heads {8r..8r+7} (whole GQA
groups: kv heads 2r, 2r+1). Compute in bf16 on the TensorEngine (fp32 PSUM
accumulation), fp32 softmax statistics. Per-core pipeline:
  1. QT = (Wq_c * scale).T @ query_b.T -> [512, L] bf16 (head-pair-major rows)
  2. kT = Wk_c.T @ kv_b.T -> [128, L]; v = kv_b @ Wv_c -> [L, 130] (+ones cols)
  3. per head-pair (g0-head, g1-head): scores^T = kT-slices.T @ QT-slices
     (two row-packed K=64 matmuls), exp, multiplicative mask (host-exp'd
     band tiles), PV^T with ones-column giving the softmax denominator in
     PSUM row 64; normalize via broadcast+reciprocal+mul; -> DRAM bounce.
  4. AllGather attnT (bf16) per head-pair over the 4-rank TP group.
  5. out_c[:, 512r:+512] = attnT_full.T @ Wo_perm_c + bo_c (PSUM-accumulated)
Host assembles [2, 2048, 2048] from per-core [2048, 512] f32 slabs.

Mask handling is input-driven: the effective additive mask (attn_mask +
key-padding) is classified on host per (q-chunk, k-tile) block as
all-masked (skip: contributes exactly 0), all-zero (no mask op), or band
(exp(mask) shipped and multiplied into exp(scores): exp(s+m) == exp(s)exp(m),
exactly 0/1 for the -1e9/0 values the reference uses).
"""

import numpy as np
import ml_dtypes
import concourse.bass as bass
import concourse.mybir as mybir
import concourse.tile as tile
from concourse import bacc
from concourse.bass_utils import run_bass_kernel_spmd

F32 = mybir.dt.float32
BF16 = mybir.dt.bfloat16
AF = mybir.ActivationFunctionType
NPBF16 = ml_dtypes.bfloat16

B, L, HID = 2, 2048, 2048
NH, D, NKV = 32, 64, 8
SCALE = 0.125
N_CORES = 8
TPR = 4          # TP ranks per batch group
NPAIR = 4        # head pairs per core (g0-head, g1-head)
LQC = 512        # Lq chunk for attention (PSUM-bank sized)
NJ = L // LQC    # 4
KT = 128         # k-position tile
NI = L // KT     # 16
NEG_THRESH = -1.0e8

_graph_cache = {}
last_results = None  # BassKernelResults of the most recent run (for test harness)


def _classify_blocks(eff_masks):
    """eff_masks: list of B arrays [L, L] (q, k). Returns (live, band_list)
    where live[j] is the ascending list of k-tiles to compute for q-chunk j and
    band_list orders the (j, i) blocks that need explicit mask values."""
    live = {}
    band_list = []
    for j in range(NJ):
        lv = []
        for i in range(NI):
            subs = [m[j * LQC:(j + 1) * LQC, i * KT:(i + 1) * KT] for m in eff_masks]
            if all((s <= NEG_THRESH).all() for s in subs):
                continue  # fully masked in every batch: contributes exactly 0
            lv.append(i)
            if not all((s == 0.0).all() for s in subs):
                band_list.append((j, i))
        live[j] = lv
    return live, band_list


def _build_graph(live_key, band_key):
    key = (live_key, band_key)
    if key in _graph_cache:
        return _graph_cache[key]

    live = {j: list(lv) for j, lv in live_key}
    band_list = list(band_key)
    band_idx = {ji: n for n, ji in enumerate(band_list)}
    nband = max(1, len(band_list))

    nc = bacc.Bacc("TRN2", target_bir_lowering=False, debug=False,
                   num_devices=N_CORES)

    qT = nc.dram_tensor("qT", [HID, L], BF16, kind="ExternalInput")
    kvT = nc.dram_tensor("kvT", [HID, L], BF16, kind="ExternalInput")
    wq = nc.dram_tensor("wq", [HID, 512], BF16, kind="ExternalInput")
    bq = nc.dram_tensor("bq", [128, 4], F32, kind="ExternalInput")
    wk = nc.dram_tensor("wk", [HID, 128], BF16, kind="ExternalInput")
    bk = nc.dram_tensor("bk", [128, 1], F32, kind="ExternalInput")
    wv = nc.dram_tensor("wv", [HID, 128], BF16, kind="ExternalInput")
    bv4 = nc.dram_tensor("bv4", [1, 512], BF16, kind="ExternalInput")
    wo = nc.dram_tensor("wo", [HID, 512], BF16, kind="ExternalInput")
    bo = nc.dram_tensor("bo", [1, 512], BF16, kind="ExternalInput")
    band = nc.dram_tensor("band", [nband, KT, LQC], BF16, kind="ExternalInput")
    ones = nc.dram_tensor("ones", [128, 128], BF16, kind="ExternalInput")
    out_ext = nc.dram_tensor("out", [L, 512], F32, kind="ExternalOutput")

    ag_in = [[nc.dram_tensor(f"ag_in{p}_{h}", [128, 1024], BF16)
              for h in range(2)] for p in range(NPAIR)]
    ag_out = [[nc.dram_tensor(f"ag_out{p}_{h}", [512, 1024], BF16)
               for h in range(2)] for p in range(NPAIR)]
    groups = [[0, 1, 2, 3], [4, 5, 6, 7]]

    with tile.TileContext(nc) as tc:
        with tc.tile_pool(name="persist", bufs=1) as persist:
            ones_sb = persist.tile([128, 128], BF16, tag="ones")
            nc.sync.dma_start(ones_sb[:], ones[:])
            bq_sb = persist.tile([128, 4], F32, tag="bq")
            nc.sync.dma_start(bq_sb[:], bq[:])
            bk_sb = persist.tile([128, 1], F32, tag="bk")
            nc.sync.dma_start(bk_sb[:], bk[:])
            bv4_sb = persist.tile([1, 512], BF16, tag="bv4")
            nc.sync.dma_start(bv4_sb[:], bv4[:])
            bo_sb = persist.tile([1, 512], BF16, tag="bo")
            nc.sync.dma_start(bo_sb[:], bo[:])
            # wk/wv loads issued on the vector queue so the sync queue can
            # start streaming qT immediately
            wk_sb = []
            wv_sb = []
            for k in range(16):
                wkt = persist.tile([128, 128], BF16, tag=f"wk{k}", name=f"wk{k}")
                nc.scalar.dma_start(wkt[:], wk[128 * k:128 * (k + 1), :])
                wk_sb.append(wkt)
                wvt = persist.tile([128, 128], BF16, tag=f"wv{k}", name=f"wv{k}")
                nc.scalar.dma_start(wvt[:], wv[128 * k:128 * (k + 1), :])
                wv_sb.append(wvt)
            kT_sb = persist.tile([128, L], BF16, tag="kT")
            v_sb = [persist.tile([128, 130], BF16, tag=f"v{t}", name=f"v{t}")
                    for t in range(NI)]
            # gathered attnT blocks, filled per-pair as AllGathers complete
            ag_sb = [persist.tile([128, L], BF16, tag=f"ag{kk}", name=f"ag{kk}")
                     for kk in range(16)]
            wo_sb = []
            for k in range(16):
                wot = persist.tile([128, 512], BF16, tag=f"wo{k}", name=f"wo{k}")
                nc.scalar.dma_start(wot[:], wo[128 * k:128 * (k + 1), :])
                wo_sb.append(wot)

            with tc.tile_pool(name="attn_era", bufs=1) as attn_era:
                QT_sb = [attn_era.tile([128, L], BF16, tag=f"qt{m}", name=f"qt{m}")
                         for m in range(4)]
                band_sb = []
                for nb in range(len(band_list)):
                    bt = attn_era.tile([KT, LQC], BF16, tag=f"band{nb}",
                                       name=f"band{nb}")
                    nc.scalar.dma_start(bt[:], band[nb])
                    band_sb.append(bt)

                # ---- KV projection
                with (
                    tc.tile_pool(name="kv_stream", bufs=3) as kv_stream,
                    tc.tile_pool(name="kv_psum", bufs=1, space="PSUM") as kv_psum,
                ):
                    for j2 in range(2):
                        psk = [kv_psum.tile([128, 512], F32, tag=f"psk{n}",
                                            name=f"psk{n}") for n in range(2)]
                        psv = [kv_psum.tile([128, 512], F32, tag=f"psv{n}",
                                            name=f"psv{n}") for n in range(2)]
                        for k in range(16):
                            ch = kv_stream.tile([128, 1024], BF16, tag="kvch")
                            nc.sync.dma_start(
                                ch[:], kvT[128 * k:128 * (k + 1),
                                           1024 * j2:1024 * (j2 + 1)])
                            for jj in range(2):
                                nc.tensor.matmul(
                                    psk[jj][:], wk_sb[k][:],
                                    ch[:, 512 * jj:512 * (jj + 1)],
                                    start=(k == 0), stop=(k == 15))
                            for grp in range(2):
                                if k == 0:
                                    nc.tensor.matmul(
                                        psv[grp][:], ones_sb[0:1, :],
                                        bv4_sb[:], start=True, stop=False,
                                        skip_group_check=True)
                                for col in range(4):
                                    tt = 4 * grp + col
                                    nc.tensor.matmul(
                                        psv[grp][:, 128 * col:128 * (col + 1)],
                                        ch[:, 128 * tt:128 * (tt + 1)],
                                        wv_sb[k][:],
                                        start=False, stop=(k == 15),
                                        skip_group_check=True)
                        for jj in range(2):
                            j = 2 * j2 + jj
                            nc.scalar.activation(
                                kT_sb[:, 512 * j:512 * (j + 1)], psk[jj][:],
                                AF.Identity, bias=bk_sb[:])
                        for grp in range(2):
                            for col in range(4):
                                t = 8 * j2 + 4 * grp + col
                                nc.scalar.copy(
                                    v_sb[t][:, 0:64],
                                    psv[grp][:, 128 * col:128 * col + 64])
                                nc.scalar.copy(
                                    v_sb[t][:, 65:129],
                                    psv[grp][:, 128 * col + 64:128 * (col + 1)])
                                nc.vector.tensor_copy(v_sb[t][:, 64:65],
                                                      ones_sb[:, 0:1])
                                nc.vector.tensor_copy(v_sb[t][:, 129:130],
                                                      ones_sb[:, 0:1])

                # ---- Q projection (N=1024 moving chunks)
                with (
                    tc.tile_pool(name="q_era", bufs=1) as q_era,
                    tc.tile_pool(name="q_stream", bufs=3) as q_stream,
                    tc.tile_pool(name="q_psum", bufs=1, space="PSUM") as q_psum,
                ):
                    wq_sb = []
                    for k in range(16):
                        wqt = q_era.tile([128, 512], BF16, tag=f"wq{k}",
                                         name=f"wq{k}")
                        nc.sync.dma_start(wqt[:], wq[128 * k:128 * (k + 1), :])
                        wq_sb.append(wqt)
                    for jp in range(2):
                        psq = [q_psum.tile([128, 512], F32, tag=f"psq{n}",
                                           name=f"psq{n}") for n in range(8)]
                        for k in range(16):
                            ch = q_stream.tile([128, 1024], BF16, tag="qch")
                            nc.sync.dma_start(
                                ch[:], qT[128 * k:128 * (k + 1),
                                          1024 * jp:1024 * (jp + 1)])
                            for m in range(4):
                                for jj in range(2):
                                    nc.tensor.matmul(
                                        psq[4 * jj + m][:],
                                        wq_sb[k][:, 128 * m:128 * (m + 1)],
                                        ch[:, 512 * jj:512 * (jj + 1)],
                                        start=(k == 0), stop=(k == 15))
                        for jj in range(2):
                            j = 2 * jp + jj
                            for m in range(4):
                                nc.scalar.activation(
                                    QT_sb[m][:, 512 * j:512 * (j + 1)],
                                    psq[4 * jj + m][:], AF.Identity,
                                    bias=bq_sb[:, m:m + 1])

                # ---- Attention per (pair, q-chunk) + per-pair AllGather
                with (
                    tc.tile_pool(name="pt_pool", bufs=3) as pt_pool,
                    tc.tile_pool(name="at_pool", bufs=3) as at_pool,
                    tc.tile_pool(name="rc_pool", bufs=2) as rc_pool,
                    tc.tile_pool(name="qk_psum", bufs=2, space="PSUM") as qk_psum,
                    tc.tile_pool(name="pv_psum", bufs=2, space="PSUM") as pv_psum,
                ):
                    pending_loads = []

                    def flush_load():
                        pr_, h_ = pending_loads.pop(0)
                        for rp in range(TPR):
                            kk = 4 * pr_ + rp
                            nc.sync.dma_start(
                                ag_sb[kk][:, 1024 * h_:1024 * (h_ + 1)],
                                ag_out[pr_][h_][128 * rp:128 * (rp + 1), :])

                    for pr in range(NPAIR):
                        for j in range(NJ):
                            lv = live[j]
                            pva = pv_psum.tile([65, 512], F32, tag="pva")
                            pvb = pv_psum.tile([65, 512], F32, tag="pvb")
                            for n, i in enumerate(lv):
                                ps = qk_psum.tile([128, 1024], F32, tag="qk")
                                nc.tensor.matmul(
                                    ps[:, 0:512],
                                    kT_sb[0:64, 128 * i:128 * (i + 1)],
                                    QT_sb[pr][0:64, 512 * j:512 * (j + 1)],
                                    start=True, stop=True,
                                    skip_group_check=True)
                                nc.tensor.matmul(
                                    ps[:, 512:1024],
                                    kT_sb[64:128, 128 * i:128 * (i + 1)],
                                    QT_sb[pr][64:128, 512 * j:512 * (j + 1)],
                                    start=True, stop=True,
                                    skip_group_check=True)
                                pt = pt_pool.tile([128, 1024], BF16, tag="pt")
                                nc.scalar.activation(pt[:], ps[:], AF.Exp)
                                if (j, i) in band_idx:
                                    bt = band_sb[band_idx[(j, i)]]
                                    nc.vector.tensor_mul(pt[:, 0:512],
                                                         pt[:, 0:512], bt[:])
                                    nc.vector.tensor_mul(pt[:, 512:1024],
                                                         pt[:, 512:1024], bt[:])
                                nc.tensor.matmul(
                                    pva[:], v_sb[i][:, 0:65], pt[:, 0:512],
                                    start=(n == 0), stop=(n == len(lv) - 1),
                                    skip_group_check=True)
                                nc.tensor.matmul(
                                    pvb[:], v_sb[i][:, 65:130], pt[:, 512:1024],
                                    start=(n == 0), stop=(n == len(lv) - 1),
                                    skip_group_check=True)
                            # normalize: denom row 64 -> copy, broadcast,
                            # reciprocal on 64 partitions, multiply
                            rsa = rc_pool.tile([1, 512], F32, tag="rsa")
                            rsb = rc_pool.tile([1, 512], F32, tag="rsb")
                            nc.vector.tensor_copy(rsa[:], pva[64:65, :])
                            nc.vector.tensor_copy(rsb[:], pvb[64:65, :])
                            rra = rc_pool.tile([1, 512], F32, tag="rra")
                            rrb = rc_pool.tile([1, 512], F32, tag="rrb")
                            nc.vector.reciprocal_approx_fast(out=rra[:], in_=rsa[:])
                            nc.vector.reciprocal_approx_fast(out=rrb[:], in_=rsb[:])
                            rba = rc_pool.tile([64, 512], F32, tag="rba")
                            rbb = rc_pool.tile([64, 512], F32, tag="rbb")
                            nc.gpsimd.partition_broadcast(rba[:], rra[:])
                            nc.gpsimd.partition_broadcast(rbb[:], rrb[:])
                            ata = at_pool.tile([64, 512], BF16, tag="ata")
                            atb = at_pool.tile([64, 512], BF16, tag="atb")
                            nc.vector.tensor_mul(ata[:], pva[0:64, :], rba[:])
                            nc.vector.tensor_mul(atb[:], pvb[0:64, :], rbb[:])
                            h, jh = j // 2, j % 2
                            nc.gpsimd.dma_start(
                                ag_in[pr][h][0:64, 512 * jh:512 * (jh + 1)],
                                ata[:])
                            nc.gpsimd.dma_start(
                                ag_in[pr][h][64:128, 512 * jh:512 * (jh + 1)],
                                atb[:])
                            if jh == 1:
                                nc.gpsimd.collective_compute(
                                    "AllGather", mybir.AluOpType.bypass,
                                    replica_groups=groups,
                                    ins=[ag_in[pr][h][:]],
                                    outs=[ag_out[pr][h][:]])
                                pending_loads.append((pr, h))
                                if len(pending_loads) > 2:
                                    flush_load()
                    while pending_loads:
                        flush_load()

                # ---- Output projection, PSUM-accumulated per Lq tile
                with (
                    tc.tile_pool(name="o_out", bufs=3) as o_out,
                    tc.tile_pool(name="o_psum", bufs=2, space="PSUM") as o_psum,
                ):
                    for t in range(NI):
                        pso = o_psum.tile([128, 512], F32, tag="pso")
                        nc.tensor.matmul(pso[:], ones_sb[0:1, :], bo_sb[:],
                                         start=True, stop=False,
                                         skip_group_check=True)
                        for kk in range(16):
                            nc.tensor.matmul(
                                pso[:], ag_sb[kk][:, 128 * t:128 * (t + 1)],
                                wo_sb[kk][:], start=False, stop=(kk == 15),
                                skip_group_check=True)
                        osb = o_out.tile([128, 512], F32, tag="osb")
                        nc.scalar.copy(osb[:], pso[:])
                        nc.sync.dma_start(out_ext[128 * t:128 * (t + 1), :],
                                          osb[:])

    nc.compile()
    _graph_cache[key] = nc
    return nc


def kernel(query, kv, Wq, bq, Wkv, bkv, Wo, bo, attn_mask, key_padding_mask):
    global last_results
    query = np.asarray(query, np.float32)
    kv = np.asarray(kv, np.float32)
    Wq = np.asarray(Wq, np.float32)
    bq = np.asarray(bq, np.float32)
    Wkv = np.asarray(Wkv, np.float32)
    bkv = np.asarray(bkv, np.float32)
    Wo = np.asarray(Wo, np.float32)
    bo = np.asarray(bo, np.float32)
    attn_mask = np.asarray(attn_mask, np.float32)
    kpm = np.asarray(key_padding_mask)

    eff = [attn_mask + np.where(kpm[b], np.float32(-1e9), np.float32(0.0))[None, :]
           for b in range(B)]
    live, band_list = _classify_blocks(eff)
    live_key = tuple((j, tuple(lv)) for j, lv in sorted(live.items()))
    band_key = tuple(band_list)

    nc = _build_graph(live_key, band_key)

    # Host-side shard prep (bf16 for all TensorEngine operands)
    qTh = [np.ascontiguousarray(query[b].T.astype(NPBF16)) for b in range(B)]
    kvTh = [np.ascontiguousarray(kv[b].T.astype(NPBF16)) for b in range(B)]
    bandh = []
    with np.errstate(over="ignore", under="ignore"):
        for b in range(B):
            if band_list:
                bandh.append(np.ascontiguousarray(np.stack(
                    [np.exp(eff[b][j * LQC:(j + 1) * LQC,
                                   i * KT:(i + 1) * KT].T)
                     for (j, i) in band_list]).astype(NPBF16)))
            else:
                bandh.append(np.zeros((1, KT, LQC), NPBF16))
    ones_h = np.ones((128, 128), NPBF16)

    Wq_h = Wq.reshape(HID, NH, D)
    bq_h = bq.reshape(NH, D)
    Wo_h = Wo.reshape(NH, D, HID)

    in_maps = []
    for c in range(N_CORES):
        b, r = c // TPR, c % TPR
        heads_q = [8 * r + pr + 4 * e for pr in range(NPAIR) for e in range(2)]
        perm_glob = [8 * rp + pr + 4 * e
                     for pr in range(NPAIR) for rp in range(TPR) for e in range(2)]
        wq_c = np.ascontiguousarray(
            (Wq_h[:, heads_q, :].reshape(HID, 512) * SCALE).astype(NPBF16))
        bq_c = np.ascontiguousarray(
            (bq_h[heads_q].reshape(512) * SCALE).reshape(4, 128).T)
        wk_c = np.ascontiguousarray(Wkv[:, 128 * r:128 * (r + 1)].astype(NPBF16))
        bk_c = np.ascontiguousarray(bkv[128 * r:128 * (r + 1)])[:, None]
        wv_c = np.ascontiguousarray(
            Wkv[:, 512 + 128 * r:512 + 128 * (r + 1)].astype(NPBF16))
        bv_c = bkv[512 + 128 * r:512 + 128 * (r + 1)]
        bv4_c = np.ascontiguousarray(np.tile(bv_c, 4).astype(NPBF16))[None, :]
        wo_c = np.ascontiguousarray(
            Wo_h[perm_glob].reshape(HID, HID)[:, 512 * r:512 * (r + 1)]
            .astype(NPBF16))
        bo_c = np.ascontiguousarray(
            bo[512 * r:512 * (r + 1)].astype(NPBF16))[None, :]
        in_maps.append({
            "qT": qTh[b], "kvT": kvTh[b],
            "wq": wq_c, "bq": bq_c,
            "wk": wk_c, "bk": bk_c,
            "wv": wv_c, "bv4": bv4_c,
            "wo": wo_c, "bo": bo_c,
            "band": bandh[b], "ones": ones_h,
        })

    last_results = run_bass_kernel_spmd(nc, in_maps, core_ids=list(range(N_CORES)))

    out = np.empty((B, L, HID), np.float32)
    for c in range(N_CORES):
        b, r = c // TPR, c % TPR
        out[b, :, 512 * r:512 * (r + 1)] = last_results.results[c]["out"]
    return out
